# revision 1
# baseline (speedup 1.0000x reference)
"""ChannelRowAttention Trainium2 kernel (v4).

Full-input contract: kernel(**inputs) takes the complete (8,256,128,128) batch
plus weights, shards batch-wise across 8 NeuronCores (one image per core), and
returns the full (8,256,128,128) output.

Per-core plan (x_img = (256,128,128), fp16 on chip; residual path fp16):
  x resident in SBUF (fp16, 64KB/partition), loaded once in 8 chunks.

  pass 1, per 4-row block, 3-deep software pipeline (kq | mid | out stages)
  to keep the PE continuously busy (PE clock ramps 1.2->2.4GHz only under
  sustained use):
    kq     = [Wk|Wq]^T . x_rows      (PE, M=128: psum parts 0:64=k, 64:128=q)
    kq -> SBUF fp16 (ACT); k replicated to parts 64:128 via SBUF->SBUF DMA
    (issued a full block ahead of its use, so DMA latency is hidden)
    attT_r = k_r^T q_r               (PE, K=64 at base 64) - attT directly,
                                     no PE transposes anywhere
    exp on ACT (fp32 psum -> bf16 SBUF; no max-subtraction: |score|<40 and
    bf16 holds e^40)
    den    = ones128^T attT_e        (PE, M=128 -> den replicated across all
                                      psum partitions in one matmul)
    inv    = 1/den                   (DVE approx reciprocal, psum -> SBUF)
    vT_r   = x_row^T . Wv^T          (PE, N=256 per row; one ACT copy)
    out_r  = vT^T . attT_e           (PE; UNNORMALIZED, fp32 psum)
    psum -> resident fp16 out via DVE scalar_tensor_tensor: multiplies by
    inv[(r,i)] (softmax normalization) in the same instruction; accum_out
    gives the per-channel sum stat. Running max stat on DVE (ping-pong).
  gate = sigmoid(W2.relu(W1.avg) + W2.relu(W1.max)): tiny fp32 PE matmuls
  pass 2, per block: final = out*(gama*gate[c]) + x -> DRAM fp16
    ch0 on DVE (scalar_tensor_tensor); ch1 on PE (diag(gscale) @ out
    += ident @ x) + ACT psum->SBUF copy, balancing engine load.
  (host casts the fp16 result back to fp32)
"""

import numpy as np
from contextlib import ExitStack

import concourse.bass as bass
from concourse import bacc
import concourse.tile as tile
from concourse import mybir
from concourse.bass_utils import run_bass_kernel_spmd

F32 = mybir.dt.float32
F16 = mybir.dt.float16
BF16 = mybir.dt.bfloat16

N, C, H, W = 8, 256, 128, 128
QK = 64
HID = 16          # SE hidden dim = C // 16
NCORES = 8
RB = 4            # rows per block
NBLK = H // RB    # 32
INV_HW = 1.0 / float(H * W)

AX = mybir.AxisListType
OP = mybir.AluOpType
AF = mybir.ActivationFunctionType


def _body(ctx: ExitStack, tc: "tile.TileContext", xh_d, wqk_d, wv_d,
          w1_d, w2_d, gama_d, id_d, y_d):
    nc = tc.nc

    const = ctx.enter_context(tc.tile_pool(name="const", bufs=1))
    stats = ctx.enter_context(tc.tile_pool(name="stats", bufs=1))
    xpool = ctx.enter_context(tc.tile_pool(name="xpool", bufs=1))
    opool = ctx.enter_context(tc.tile_pool(name="opool", bufs=1))
    work = ctx.enter_context(tc.tile_pool(name="work", bufs=3))
    finpool = ctx.enter_context(tc.tile_pool(name="fin", bufs=4))
    # PSUM budget (8 banks): kq 1 | kq2 1 | attT/den/p2 shared tag 2 | vt 2 | out 2
    psK = ctx.enter_context(tc.tile_pool(name="psK", bufs=1, space="PSUM"))
    psT = ctx.enter_context(tc.tile_pool(name="psT", bufs=2, space="PSUM"))
    psV = ctx.enter_context(tc.tile_pool(name="psV", bufs=1, space="PSUM"))
    psO = ctx.enter_context(tc.tile_pool(name="psO", bufs=1, space="PSUM"))

    # ---- constants -------------------------------------------------------
    # wqk first on the sync queue (warm-up + block 0 depend on it); the x
    # chunks go through the gpsimd-submitted queue so neither blockades the
    # other. Weights not needed until later are submitted last.
    wqk_sb = const.tile([128, 2, 128], F16)
    nc.sync.dma_start(out=wqk_sb, in_=wqk_d[:, :].rearrange("(kc p) m -> p kc m", p=128))
    x_sb = xpool.tile([128, 2, H, W], F16)
    bounds = [0, 4, 8, 16, 32, 64, 96, 128]
    for lo, hi in zip(bounds[:-1], bounds[1:]):
        nc.gpsimd.dma_start(
            out=x_sb[:, :, lo:hi, :],
            in_=xh_d[:, lo:hi, :].rearrange("(kc p) h w -> p kc h w", p=128),
        )
    wv_sb = const.tile([128, 2, C], F16)
    nc.sync.dma_start(out=wv_sb, in_=wv_d[:, :].rearrange("(kc p) m -> p kc m", p=128))
    w1_sb = const.tile([128, 2, HID], F32)
    nc.sync.dma_start(out=w1_sb, in_=w1_d[:, :].rearrange("(kc p) m -> p kc m", p=128))
    w2_sb = const.tile([HID, 2, 128], F32)
    nc.sync.dma_start(out=w2_sb, in_=w2_d[:, :].rearrange("k (mc m) -> k mc m", m=128))
    gama_sb = const.tile([128, 1], F32)
    nc.sync.dma_start(out=gama_sb, in_=gama_d[:, :].to_broadcast([128, 1]))
    ident = const.tile([128, 128], F16)
    nc.sync.dma_start(out=ident, in_=id_d[:, :])
    ones_sb = const.tile([128, 128], BF16)
    nc.vector.memset(ones_sb, 1.0)
    gscale = const.tile([128, 2], F32)      # gama * sigmoid(gate), filled later
    diag1 = const.tile([128, 128], F16)     # diag(gscale[:,1]), filled later

    # ---- resident attention output ---------------------------------------
    ob_all = opool.tile([128, 2, H, W], F16)

    sums_acc = stats.tile([128, 2, NBLK], F32)
    nc.vector.memset(sums_acc, 0.0)
    mxa = stats.tile([128, 2, 2 * RB, W], F16)
    nc.vector.memset(mxa, -60000.0)
    mxb = stats.tile([128, 2, 2 * RB, W], F16)

    # ---- pass 1: 3-deep pipeline -----------------------------------------
    kq_sbs = [None] * NBLK
    k2_sbs = [None] * NBLK
    attT_es = [None] * NBLK
    inv_bs = [None] * NBLK
    vt_sbs = [None] * NBLK
    out_pss = [None] * NBLK

    def stage_kq(b):
        h0 = b * RB
        xr = x_sb[:, :, h0:h0 + RB, :]
        kq_ps = psK.tile([128, RB, W], F32, tag="kq")
        for kc in (0, 1):
            nc.tensor.matmul(
                out=kq_ps[:, :, :].rearrange("p r w -> p (r w)"),
                lhsT=wqk_sb[:, kc, :],
                rhs=xr[:, kc, :, :].rearrange("p r w -> p (r w)"),
                start=(kc == 0), stop=(kc == 1),
            )
        kq_sb = work.tile([128, RB, W], F16, tag="kq_sb")
        nc.scalar.copy(out=kq_sb, in_=kq_ps)
        # replicate k to parts 64:128. Early blocks recompute via matmul
        # (the single DMA queue is busy streaming x in); once the x loads
        # have drained, a cheap SBUF->SBUF DMA does it instead.
        k2_sb = work.tile([128, RB, W], F16, tag="k2_sb")
        if b < 12:
            k2_ps = psK.tile([128, RB, W], F32, tag="kq2")
            for kc in (0, 1):
                nc.tensor.matmul(
                    out=k2_ps[64:128, :, :].rearrange("p r w -> p (r w)"),
                    lhsT=wqk_sb[:, kc, 0:64],
                    rhs=xr[:, kc, :, :].rearrange("p r w -> p (r w)"),
                    start=(kc == 0), stop=(kc == 1),
                )
            nc.scalar.copy(out=k2_sb[64:128, :, :], in_=k2_ps[64:128, :, :])
        else:
            nc.sync.dma_start(out=k2_sb[64:128, :, :], in_=kq_sb[0:64, :, :])
        kq_sbs[b] = kq_sb
        k2_sbs[b] = k2_sb

    def stage_mid(b):
        h0 = b * RB
        kq_sb, k2_sb = kq_sbs[b], k2_sbs[b]

        # attT[j, i] per row (K=64 at base partition 64)
        attT_ps = psT.tile([128, RB, W], F32, tag="attT")
        for r in range(RB):
            nc.tensor.matmul(
                out=attT_ps[:, r, :],
                lhsT=k2_sb[64:128, r, :],
                rhs=kq_sb[64:128, r, :],
                start=True, stop=True,
            )
        attT_e = work.tile([128, RB, W], BF16, tag="attT_e")
        nc.scalar.activation(out=attT_e, in_=attT_ps, func=AF.Exp)
        attT_es[b] = attT_e

        # vT per row (w on partitions, c on free)
        vt_ps = psV.tile([128, RB, C], F32, tag="vt")
        for r in range(RB):
            for kc in (0, 1):
                nc.tensor.matmul(
                    out=vt_ps[:, r, :],
                    lhsT=x_sb[:, kc, h0 + r, :],
                    rhs=wv_sb[:, kc, :],
                    start=(kc == 0), stop=(kc == 1),
                )
        vt_sb = work.tile([128, RB, C], BF16, tag="vt_sb")
        nc.scalar.copy(out=vt_sb, in_=vt_ps)
        vt_sbs[b] = vt_sb

        # softmax denominator, replicated across partitions in one matmul;
        # shares the attT psum tag (ping-pong within the 2 bufs)
        den_ps = psT.tile([128, RB * W], F32, tag="attT")
        nc.tensor.matmul(
            out=den_ps,
            lhsT=ones_sb,
            rhs=attT_e[:, :, :].rearrange("p r w -> p (r w)"),
            start=True, stop=True,
        )
        inv_b = work.tile([128, RB, W], F32, tag="inv_b")
        nc.vector.reciprocal_approx_fast(
            out=inv_b[:, :, :].rearrange("p r w -> p (r w)"), in_=den_ps)
        inv_bs[b] = inv_b

    def stage_out(b):
        h0 = b * RB
        attT_e, vt_sb, inv_b = attT_es[b], vt_sbs[b], inv_bs[b]

        # out = vT^T @ attT_e -> (c, i), unnormalized fp32 in psum
        out_ps = psO.tile([128, 2, RB, W], F32, tag="out")
        for r in range(RB):
            for ch in (0, 1):
                nc.tensor.matmul(
                    out=out_ps[:, ch, r, :],
                    lhsT=vt_sb[:, r, 128 * ch:128 * (ch + 1)],
                    rhs=attT_e[:, r, :],
                    start=True, stop=True,
                )
        # psum -> resident fp16, normalizing by inv[(r,i)]; accum -> sums
        for ch in (0, 1):
            nc.vector.scalar_tensor_tensor(
                out=ob_all[:, ch, h0:h0 + RB, :],
                in0=out_ps[:, ch], scalar=1.0, in1=inv_b,
                op0=OP.mult, op1=OP.mult,
                accum_out=sums_acc[:, ch, b:b + 1])
        # running max stat (DVE), ping-pong accumulators; batched over
        # block pairs to halve instruction overhead
        if b % 2 == 1:
            lo = (b - 1) * RB
            src, dst = (mxa, mxb) if (b // 2) % 2 == 0 else (mxb, mxa)
            nc.vector.tensor_tensor(
                out=dst, in0=src, in1=ob_all[:, :, lo:h0 + RB, :], op=OP.max)

    # PE warm-up: keep the tensor engine busy while x streams in, so the
    # PE clock has ramped by the time block 0 issues (and block 0 need not
    # wait for DMA with an idle PE)
    warm_ps = psK.tile([128, RB, W], F32, tag="kq")
    for _ in range(10):
        for kc in (0, 1):
            nc.tensor.matmul(
                out=warm_ps[:, 0, :],
                lhsT=wqk_sb[:, kc, :],
                rhs=wqk_sb[:, kc, :],
                start=(kc == 0), stop=(kc == 1),
            )

    for i in range(NBLK + 2):
        if i >= 2:
            stage_out(i - 2)
        if 1 <= i <= NBLK:
            stage_mid(i - 1)
        if i < NBLK:
            stage_kq(i)

    # ---- gate ------------------------------------------------------------
    mxfin = mxa if (NBLK // 2) % 2 == 0 else mxb
    mx = stats.tile([128, 2], F32)
    nc.vector.tensor_reduce(out=mx, in_=mxfin, axis=AX.XY, op=OP.max)

    mlp_in = stats.tile([128, 2, 2], F32)
    sums = stats.tile([128, 2], F32)
    nc.vector.tensor_reduce(out=sums, in_=sums_acc, axis=AX.X, op=OP.add)
    nc.vector.tensor_scalar_mul(out=mlp_in[:, :, 0], in0=sums, scalar1=INV_HW)
    nc.vector.tensor_copy(out=mlp_in[:, :, 1], in_=mx)

    h_ps = psT.tile([HID, 2], F32, tag="attT")
    for kc in (0, 1):
        nc.tensor.matmul(
            out=h_ps,
            lhsT=w1_sb[:, kc, :],
            rhs=mlp_in[:, kc, :],
            start=(kc == 0), stop=(kc == 1),
        )
    hr = stats.tile([HID, 2], F32)
    nc.vector.tensor_scalar_max(out=hr, in0=h_ps, scalar1=0.0)
    g_ps = psT.tile([128, 2, 2], F32, tag="attT")
    for mc in (0, 1):
        nc.tensor.matmul(
            out=g_ps[:, mc, :],
            lhsT=w2_sb[:, mc, :],
            rhs=hr,
            start=True, stop=True,
        )
    zt = stats.tile([128, 2], F32)
    nc.vector.tensor_reduce(out=zt, in_=g_ps, axis=AX.X, op=OP.add)
    th = stats.tile([128, 2], F32)
    nc.scalar.activation(out=th, in_=zt, func=AF.Tanh, scale=0.5)
    u = stats.tile([128, 2], F32)
    nc.vector.tensor_scalar_add(out=u, in0=th, scalar1=1.0)
    # gscale = gama * sigmoid(z) = gama * 0.5 * (1 + tanh(z/2))
    nc.vector.tensor_scalar(
        out=gscale, in0=u, scalar1=gama_sb, scalar2=0.5, op0=OP.mult, op1=OP.mult)
    # diag(gscale[:,1]) for the PE-side pass-2 channel
    nc.vector.tensor_scalar_mul(out=diag1, in0=ident, scalar1=gscale[:, 1:2])

    # ---- pass 2: final = out*gscale[c] + x -> DRAM (fp16) ----------------
    # 16-row blocks (4KB DMA runs), output split across two DMA queues;
    # ch0 on DVE, ch1 on PE+ACT
    RB2 = 8
    for j in range(H // RB2):
        h0 = j * RB2
        fin = finpool.tile([128, 2, RB2, W], F16, tag="fin")
        nc.vector.scalar_tensor_tensor(
            out=fin[:, 0], in0=ob_all[:, 0, h0:h0 + RB2, :],
            scalar=gscale[:, 0:1], in1=x_sb[:, 0, h0:h0 + RB2, :],
            op0=OP.mult, op1=OP.add)
        for q in range(2):
            hh = h0 + q * 4
            if (2 * j + q) % 3 == 0:
                p2_ps = psT.tile([128, 4 * W], F32, tag="attT")
            elif (2 * j + q) % 3 == 1:
                p2_ps = psO.tile([128, 4 * W], F32, tag="out")
            else:
                p2_ps = psV.tile([128, 4 * W], F32, tag="vt")
            nc.tensor.matmul(
                out=p2_ps,
                lhsT=diag1,
                rhs=ob_all[:, 1, hh:hh + 4, :].rearrange("p r w -> p (r w)"),
                start=True, stop=False,
            )
            nc.tensor.matmul(
                out=p2_ps,
                lhsT=ident,
                rhs=x_sb[:, 1, hh:hh + 4, :].rearrange("p r w -> p (r w)"),
                start=False, stop=True,
            )
            nc.scalar.copy(
                out=fin[:, 1, q * 4:q * 4 + 4, :]
                    .rearrange("p r w -> p (r w)"),
                in_=p2_ps)
        nc.sync.dma_start(
            out=y_d[:, h0:h0 + RB2, :].rearrange("(kc p) h w -> p kc h w", p=128),
            in_=fin,
        )


def build_nc() -> bass.Bass:
    nc = bacc.Bacc()
    xh_d = nc.dram_tensor("xh", [C, H, W], F16, kind="ExternalInput")
    wqk_d = nc.dram_tensor("wqkT", [C, 128], F16, kind="ExternalInput")
    wv_d = nc.dram_tensor("wvT", [C, C], F16, kind="ExternalInput")
    w1_d = nc.dram_tensor("w1T", [C, HID], F32, kind="ExternalInput")
    w2_d = nc.dram_tensor("w2T", [HID, C], F32, kind="ExternalInput")
    gama_d = nc.dram_tensor("gama", [1, 1], F32, kind="ExternalInput")
    id_d = nc.dram_tensor("ident", [128, 128], F16, kind="ExternalInput")
    y_d = nc.dram_tensor("out", [C, H, W], F16, kind="ExternalOutput")

    with tile.TileContext(nc) as tc:
        with ExitStack() as ctx:
            _body(ctx, tc, xh_d[:, :, :], wqk_d[:, :],
                  wv_d[:, :], w1_d[:, :], w2_d[:, :], gama_d[:, :],
                  id_d[:, :], y_d[:, :, :])
    nc.compile()
    return nc


_NC_CACHE = {}


def _get_nc():
    if "nc" not in _NC_CACHE:
        _NC_CACHE["nc"] = build_nc()
    return _NC_CACHE["nc"]


def _make_in_maps(x, Wq, Wk, Wv, W1, W2, gama):
    wqkT = np.ascontiguousarray(
        np.concatenate([Wk, Wq], axis=0).T.astype(np.float16))
    wvT = np.ascontiguousarray(Wv.T.astype(np.float16))
    w1T = np.ascontiguousarray(W1.T.astype(np.float32))
    w2T = np.ascontiguousarray(W2.T.astype(np.float32))
    g = np.asarray(gama, dtype=np.float32).reshape(1, 1)
    ident = np.eye(128, dtype=np.float16)
    maps = []
    for i in range(NCORES):
        maps.append({
            "xh": np.ascontiguousarray(x[i].astype(np.float16)),
            "wqkT": wqkT, "wvT": wvT, "w1T": w1T, "w2T": w2T, "gama": g,
            "ident": ident,
        })
    return maps


def run(x, Wq, Wk, Wv, W1, W2, gama, trace=False):
    nc = _get_nc()
    in_maps = _make_in_maps(x, Wq, Wk, Wv, W1, W2, gama)
    res = run_bass_kernel_spmd(nc, in_maps, core_ids=list(range(NCORES)),
                               trace=trace)
    y = np.stack([res.results[i]["out"].astype(np.float32)
                  for i in range(NCORES)], axis=0)
    return y, res


def kernel(x, Wq, Wk, Wv, W1, W2, gama):
    x = np.asarray(x); Wq = np.asarray(Wq); Wk = np.asarray(Wk)
    Wv = np.asarray(Wv); W1 = np.asarray(W1); W2 = np.asarray(W2)
    gama = np.asarray(gama)
    y, _ = run(x, Wq, Wk, Wv, W1, W2, gama, trace=False)
    return y.astype(np.float32)

